# revision 56
# baseline (speedup 1.0000x reference)
# Bass/Tile Trainium2 kernel for nn_Attention_48816598286380.
#
# Reference computation (B=4, N=512, M=8192, Hq=512, Ck=256, H=8, D=64):
#   q = x @ Wq;  k,v = split(context @ Wkv);  per-head softmax(q k^T / sqrt(D)) v
#   out = attn_out @ Wo + bo
#
# Sharding: 8 cores = 4 batches x 2 head-groups (4 heads each).  Each core
# computes its batch's attention for its 4 heads plus the partial output
# projection over those heads; the host sums the two partial projections per
# batch (bo is split half/half so the sum carries the full bias).
#
# The kernel is Activation-engine bound: every score element passes through
# ACT exp exactly once (16.8M elements/core = 131072 rows at 128 lanes,
# ~0.85ns/row + ~0.19us/instr overhead), so the design keeps ACT dense from
# t~8us and pulls all other engines under it:
#   all matmul inputs are bf16 (host-converted; fp32 accumulation in PSUM)
#     which halves every input DMA at unchanged PE cost
#   scoresT[m, (h n)] = kT(m-tile).T @ qT  (two heads per PSUM tile)
#   E = exp(scoresT/8) on ACT, PSUM -> SBUF bf16
#   AV flipped: acc[n-tile, d_aug] += E-tile.T @ v_aug  (bf16, J=65; v_aug =
#     [v | ones] so the softmax denominator falls out of column 64).  AV
#     lags QK by one m-tile so the exp wait never blocks the next QK, and
#     all AV matmuls accumulate onto DVE-zeroed PSUM: interleaved
#     start=True groups within one PSUM bank corrupt earlier unstopped
#     groups on real hardware.
#   tail: per-head reciprocal + per-n-tile scale (DVE), PE transpose via
#     identity back to [d, n]; the pair-1 projection accumulates onto PSUM
#     seeded with pair-0's result via an identity matmul, the (idle) ACT
#     engine copies PSUM->SBUF bf16, and the host sums the two partials.
# kT pair 1 stays resident in SBUF (bf16, 16KB/partition) so pass 1 has no
# production or ct DMA at all.

import numpy as np

B, N, M = 4, 512, 8192
QUERY_DIM, INPUT_DIM = 512, 256
HEADS, DIM_HEAD = 8, 64
ATT_DIM = HEADS * DIM_HEAD  # 512
HPC = 4          # heads per core
N_CORES = 8
# chunk schedule: two small chunks first so the first scores/exp start
# as early as possible, then full-size chunks
CHUNKS = [(0, 512), (512, 512)] + [(m0, 1024) for m0 in range(1024, M, 1024)]
MCHUNK = 1024    # max chunk size (pool slot size)
NMI = M // 128   # 64 m-tiles per pass
SCALE = DIM_HEAD ** -0.5

# Pass-1 is ACT-throughput-bound (no production to hide behind), so DVE_EXP
# of its 64 exp tiles run on the Vector engine via a calibrated Schraudolph
# pair: s1 = i16(round(A*s + B1)) (DVE converts f32->int round-to-nearest),
# s2 = s1 + 64 (exactly the half-octave-offset second factor),
# E = bf16(s1.bf16 * s2.bf16).  Ripple ~1% rms, mean calibrated to exp;
# softmax cancels most of it (measured end-to-end rel-err cost ~2e-4).
DVE_EXP = 19
DVE_MIN_MI = 6       # keep the pre-issued boundary tiles on ACT
# pass-0 DVE-exp tiles: one per 1024-chunk, at the chunk's second-to-last
# visit (the production-copy queue on DVE is empty there)
DVE0_TILES = frozenset(range(13, NMI, 8))
LOG2E = float(np.log2(np.e))
SCH_A = 128.0 * LOG2E * SCALE / 2.0
SCH_B1 = 128.0 * (127.0 - 0.30755)

_CACHE = {}


def _build_nc():
    import concourse.bacc as bacc
    import concourse.bass as bass
    import concourse.mybir as mybir
    import concourse.tile as tile

    f32 = mybir.dt.float32
    f32r = mybir.dt.float32r
    bf16 = mybir.dt.bfloat16
    i16 = mybir.dt.int16
    EXP = mybir.ActivationFunctionType.Exp
    ADD = mybir.AluOpType.add
    MULT = mybir.AluOpType.mult

    # spread the DVE-exp'd pass-1 tiles evenly over mi in [DVE_MIN_MI, NMI)
    dve_tile = {DVE_MIN_MI + (i * (NMI - DVE_MIN_MI)) // DVE_EXP
                for i in range(DVE_EXP)}

    nc = bacc.Bacc(None, target_bir_lowering=False)

    ct = nc.dram_tensor("ct", [INPUT_DIM, M], bf16, kind="ExternalInput")  # context[b].T
    xt = nc.dram_tensor("xt", [QUERY_DIM, N], bf16, kind="ExternalInput")  # x[b].T
    wq = nc.dram_tensor("wq", [QUERY_DIM, HPC * DIM_HEAD], bf16, kind="ExternalInput")
    wk = nc.dram_tensor("wk", [INPUT_DIM, HPC * DIM_HEAD], bf16, kind="ExternalInput")
    wv = nc.dram_tensor("wv", [INPUT_DIM, HPC * DIM_HEAD], bf16, kind="ExternalInput")
    wo = nc.dram_tensor("wo", [DIM_HEAD, HPC, QUERY_DIM], f32r, kind="ExternalInput")
    bo2 = nc.dram_tensor("bo2", [1, QUERY_DIM], f32, kind="ExternalInput")  # bo / 2
    ident = nc.dram_tensor("ident", [128, 128], bf16, kind="ExternalInput")
    out = nc.dram_tensor("out", [N, QUERY_DIM], bf16, kind="ExternalOutput")

    ct_r = ct[:, :].rearrange("(t p) m -> p t m", p=128)    # [128, 2, M]
    xt_r = xt[:, :].rearrange("(t p) n -> p t n", p=128)    # [128, 4, N]
    wq_r = wq[:, :].rearrange("(t p) d -> p t d", p=128)    # [128, 4, 256]
    wk_r = wk[:, :].rearrange("(t p) d -> p t d", p=128)    # [128, 2, 256]
    wv_r = wv[:, :].rearrange("(t p) d -> p t d", p=128)    # [128, 2, 256]
    out_r = out[:, :].rearrange("(t p) f -> p t f", p=128)  # [128, 4, 512]

    with tile.TileContext(nc) as tc:
        with (
            tc.tile_pool(name="const", bufs=1) as cp,
            tc.tile_pool(name="ctp", bufs=3) as ctp,
            tc.tile_pool(name="ktp", bufs=2) as ktp,
            tc.tile_pool(name="ep", bufs=12) as ep,
            tc.tile_pool(name="sp1", bufs=3) as sp1,
            tc.tile_pool(name="sp2", bufs=3) as sp2,
            tc.tile_pool(name="scp", bufs=3, space="PSUM") as scp,
            tc.tile_pool(name="accp", bufs=1, space="PSUM") as accp,
        ):
            # ---- constants ----
            xt_sb = cp.tile([128, 4, N], bf16)
            wq_sb = cp.tile([128, 4, HPC * DIM_HEAD], bf16)
            wk_sb = cp.tile([128, 2, HPC * DIM_HEAD], bf16)
            wv_sb = cp.tile([128, 2, HPC * DIM_HEAD], bf16)
            wo_sb = cp.tile([DIM_HEAD, HPC, QUERY_DIM], f32r)
            bo_sb = cp.tile([1, QUERY_DIM], f32)
            bo_bc = cp.tile([128, QUERY_DIM], f32)
            ident_sb = cp.tile([128, 128], bf16)
            qt_sb = cp.tile([128, 2, N], bf16)
            # v for all 4 heads, all of M, with a ones column per head:
            # [128 (m within tile), m-tile, head, 64 v | 1 one]  (bf16)
            v_full = cp.tile([128, NMI, HPC, DIM_HEAD + 1], bf16)
            kt_f1 = cp.tile([128, M], bf16)               # pair-1 kT, resident
            stack_sb = cp.tile([DIM_HEAD, HPC, N], f32r)  # normalized attn outT
            recip_sb = cp.tile([128, 2, 4], f32)          # per pass-head, n-tile
            norm_sb = cp.tile([128, 2, 4, DIM_HEAD], bf16)  # normalized [n,d]
            out0_sb = cp.tile([128, 4, QUERY_DIM], bf16)  # pair-0 proj + bias
            out_sb = cp.tile([128, 4, QUERY_DIM], bf16)

            # prologue DMAs, interleaved so the qT path (wq+xt, the longer
            # pole) and the kT path (wk+ct0) both finish as early as
            # possible; xt is split per t-tile so the qT matmuls accumulate
            # as tiles land.
            nc.sync.dma_start(out=wq_sb[:, :, 0:128], in_=wq_r[:, :, 0:128])
            nc.sync.dma_start(out=xt_sb[:, 0:2, :], in_=xt_r[:, 0:2, :])
            nc.sync.dma_start(out=wk_sb[:], in_=wk_r)
            nc.sync.dma_start(out=xt_sb[:, 2:4, :], in_=xt_r[:, 2:4, :])

            # PE warm-up: the HAM clock gate holds the PE at 1.2 GHz until
            # ~3.4 us of sustained activity.  Run throwaway matmuls on a
            # zeroed tile while the prologue DMAs are in flight.
            warm_sb = cp.tile([128, 64], f32)
            nc.vector.memset(warm_sb[:], 0.0)
            # acc tiles are padded to [128, 4, 128] (exactly one 2KB PSUM
            # bank) so each [:, nt, 0:65] accumulation region stays inside
            # one bank.
            warm_ps = accp.tile([128, 4, 128], f32, tag="acc0",
                                name="warm_ps")
            for w in range(15):
                nc.tensor.matmul(
                    warm_ps[0:64, 0, 0:64], lhsT=warm_sb[:], rhs=warm_sb[:],
                    start=True, stop=True, skip_group_check=True,
                )

            kt_of = {}
            pre = {}   # (p, mi) -> e_t issued ahead of schedule
            dve_stash = {}

            def produce_chunk(mc):
                """DMA chunk mc of contextT; kT for pair 0 goes to rotating
                chunk tiles, pair 1 to the resident kt_f1, v (all 4 heads)
                to v_full.  Returns emitter closures so production
                interleaves with attention tiles."""
                m0, mlen = CHUNKS[mc]
                ct_t = ctp.tile([128, 2, MCHUNK], bf16, tag="ct",
                                name=f"ct{mc}")
                ct_dma = nc.sync.dma_start(
                    out=ct_t[:, :, 0:mlen], in_=ct_r[:, :, m0:m0 + mlen]
                )
                if mc >= 1:
                    # keep the small prologue DMAs ahead of the chunk stream
                    for d in late_dmas:
                        tile.add_dep_helper(ct_dma.ins, d.ins, sync=False,
                                            reason="prologue before ct stream")
                kt_t = ktp.tile([128, MCHUNK], bf16, tag="kt", name=f"kt{mc}")
                for mi in range(m0 // 128, (m0 + mlen) // 128):
                    kt_of[mi] = (kt_t, mi * 128 - m0)
                halves = mlen // 512

                def kt_group(pp):
                    def go():
                        kt_ps = scp.tile([128, 1024], f32, tag="sc",
                                         name=f"ktps{pp}{mc}")
                        for h2 in range(halves):
                            for t in range(2):
                                nc.tensor.matmul(
                                    kt_ps[:, h2 * 512:(h2 + 1) * 512],
                                    lhsT=wk_sb[:, t, pp * 128:(pp + 1) * 128],
                                    rhs=ct_t[:, t, h2 * 512:(h2 + 1) * 512],
                                    start=(t == 0), stop=(t == 1),
                                    skip_group_check=True,
                                )
                        dst = (kt_t[:, 0:mlen] if pp == 0 else
                               kt_f1[:, m0:m0 + mlen])
                        nc.vector.tensor_copy(dst, kt_ps[:, 0:mlen])
                    return go

                def v_group(s4):
                    def go():
                        v_ps = scp.tile([128, 1024], f32, tag="sc",
                                        name=f"vps{mc}{s4}")
                        for q in range(4):
                            s = s4 * 4 + q
                            for t in range(2):
                                nc.tensor.matmul(
                                    v_ps[:, q * 256:(q + 1) * 256],
                                    lhsT=ct_t[:, t, s * 128:(s + 1) * 128],
                                    rhs=wv_sb[:, t, :],
                                    start=(t == 0), stop=(t == 1),
                                    skip_group_check=True,
                                )
                        nc.vector.tensor_copy(
                            v_full[:, m0 // 128 + s4 * 4:
                                   m0 // 128 + s4 * 4 + 4, :, 0:DIM_HEAD],
                            v_ps[:].rearrange("p (s h d) -> p s h d", s=4, h=HPC),
                        )
                    return go

                # order: pair-0 kT first (needed immediately), v next (needed
                # by AV shortly after), pair-1 kT last (pass 1 only).  For
                # chunks >= 2 the pair-1 kT production is DEFERRED into pass
                # 1 (produce_kt1): pass 0 is PE-bound, pass 1 has PE slack.
                ops = [kt_group(0)]
                ops += [v_group(s4) for s4 in range(halves)]
                if mc < 2:
                    ops.append(kt_group(1))
                return ops

            def produce_kt1(mc):
                """Pass-1 deferred pair-1 kT production for chunk mc: re-DMA
                the ct chunk (DMA is idle in pass 1) and emit per-512-col
                closures so the borrowed score-ring slots are held briefly."""
                m0, mlen = CHUNKS[mc]
                ct_t = ctp.tile([128, 2, MCHUNK], bf16, tag="ct",
                                name=f"ct1_{mc}")
                nc.sync.dma_start(out=ct_t[:, :, 0:mlen],
                                  in_=ct_r[:, :, m0:m0 + mlen])

                def kt1_half(h2):
                    def go():
                        kt_ps = scp.tile([128, 1024], f32, tag="sc",
                                         name=f"ktps1_{mc}_{h2}")
                        for t in range(2):
                            nc.tensor.matmul(
                                kt_ps[:, 0:512],
                                lhsT=wk_sb[:, t, 128:256],
                                rhs=ct_t[:, t, h2 * 512:(h2 + 1) * 512],
                                start=(t == 0), stop=(t == 1),
                                skip_group_check=True,
                            )
                        dst = kt_f1[:, m0 + h2 * 512:m0 + (h2 + 1) * 512]
                        nc.vector.tensor_copy(dst, kt_ps[:, 0:512])
                    return go

                return [kt1_half(h2) for h2 in range(mlen // 512)]

            def qk_exp(p, mi):
                sc = scp.tile([128, 1024], f32, tag="sc", name=f"sc{p}{mi}")
                ks, off = kt_of[mi] if p == 0 else (kt_f1, mi * 128)
                ks = ks[:, off:off + 128]
                # two heads in one PE pass via row tiling
                nc.tensor.matmul(sc[:, 0:512], lhsT=ks[0:64, :],
                                 rhs=qt_sb[0:64, p, :], start=True, stop=True)
                nc.tensor.matmul(sc[:, 512:1024], lhsT=ks[64:128, :],
                                 rhs=qt_sb[64:128, p, :], start=True, stop=True)
                e_t = ep.tile([128, 1024], bf16, tag="e", name=f"e{p}{mi}")
                if (p == 1 and mi in dve_tile) or (p == 0 and mi in DVE0_TILES):
                    s1 = sp1.tile([128, 1024], i16, tag="s1", name=f"s1{p}_{mi}")
                    s2 = sp2.tile([128, 1024], i16, tag="s2", name=f"s2{p}_{mi}")
                    nc.vector.tensor_scalar(s1[:], sc[:], SCH_A, SCH_B1,
                                            MULT, ADD)
                    nc.vector.tensor_scalar(s2[:], s1[:], 64, None, ADD)
                    nc.vector.tensor_tensor(e_t[:], s1[:].bitcast(bf16),
                                            s2[:].bitcast(bf16), MULT)
                else:
                    nc.scalar.activation(e_t[:], sc[:], EXP, scale=SCALE)
                return e_t

            def av(p, mi, e_t, acc):
                # flipped AV: weights = E n-tile (full 128 rows), stream the
                # 65 v_aug columns
                for h2 in range(2):
                    for nt in range(4):
                        nc.tensor.matmul(
                            acc[h2][:, nt, 0:DIM_HEAD + 1],
                            lhsT=e_t[:, h2 * 512 + nt * 128:
                                     h2 * 512 + (nt + 1) * 128],
                            rhs=v_full[:, mi, 2 * p + h2, :],
                            start=False, stop=(mi == NMI - 1),
                            skip_group_check=True,
                        )

            def pass_tail(p, acc):
                """acc[h2] is [128 n(tile), 4 nt, 65] raw numerators with the
                denominator in column 64.  Normalize rows with one reciprocal
                and one broadcast multiply per head, transpose each
                [128 n, 64 d] block back to [d, n] on the PE, and return the
                two [64, N] PSUM tiles.  Pass 0 parks the transposed tiles in
                the (just-read) acc banks so the ring stays free for pass 1;
                pass 1 parks them in ring slot 2 (free after the last exp)."""
                tps = []
                for h2 in range(2):
                    tp_ps = accp.tile([DIM_HEAD, N], bf16,
                                      tag=f"acc{h2}", name=f"tp{p}{h2}")
                    nc.vector.reciprocal(
                        recip_sb[:, h2, :],
                        acc[h2][:, :, DIM_HEAD],
                    )
                    for nt in range(4):
                        nc.vector.tensor_scalar_mul(
                            norm_sb[:, h2, nt, :],
                            acc[h2][:, nt, 0:DIM_HEAD],
                            recip_sb[:, h2, nt:nt + 1],
                        )
                        nc.tensor.transpose(
                            tp_ps[:, nt * 128:(nt + 1) * 128],
                            norm_sb[:, h2, nt, :],
                            ident_sb[:],
                        )
                    tps.append(tp_ps)
                return tps

            # chunk-0 context DMA goes out right behind wk; the second half
            # of wq (pair 1) follows
            chunk0 = produce_chunk(0)
            nc.sync.dma_start(out=wq_sb[:, :, 128:256], in_=wq_r[:, :, 128:256])

            # late prologue (not needed until mid-kernel)
            late_dmas = []
            late_dmas.append(nc.sync.dma_start(out=wv_sb[:], in_=wv_r))
            # ones column of v_aug: memset a [128, 1] column, then one
            # broadcast-copy into the strided ones slots
            ones_col = cp.tile([128, 1], bf16)
            nc.vector.memset(ones_col[:], 1.0)
            _oc, _vdst = bass.broadcast_tensor_aps(
                ones_col[:, :], v_full[:, :, :, DIM_HEAD].rearrange(
                    "p s h -> p (s h)")[:, None, :].rearrange("p o q -> p (o q)")
            )
            nc.vector.tensor_copy(_vdst, _oc)

            # qT pair 0 (matmuls + copy) first, then chunk-0 kT (whose ct
            # lands slightly later), then qT pair 1; the DVE copy order
            # (qt-p0, kt, qt-p1) matches what the first QK needs.
            q_ps = scp.tile([128, 1024], f32, tag="sc", name="q_ps")

            def q_pair(p):
                for t in range(4):
                    nc.tensor.matmul(
                        q_ps[:, p * 512:(p + 1) * 512],
                        lhsT=wq_sb[:, t, p * 128:(p + 1) * 128],
                        rhs=xt_sb[:, t, :],
                        start=(t == 0), stop=(t == 3),
                        skip_group_check=True,
                    )
                nc.vector.tensor_copy(
                    qt_sb[:, p, :], q_ps[:, p * 512:(p + 1) * 512])

            q_pair(0)
            chunk0[0]()
            chunk0 = chunk0[1:]
            q_pair(1)

            # partial projection for pair 0 (+ bias) overlaps pass 1
            def proj_pair0(g):
                if True:
                    pr0 = scp.tile([128, 1024], f32, tag="sc", name=f"pr0{g}")
                    for j in range(2):
                        nt = g * 2 + j
                        for h in range(2):
                            nc.tensor.matmul(
                                pr0[:, j * 512:(j + 1) * 512],
                                lhsT=stack_sb[:, h, nt * 128:(nt + 1) * 128],
                                rhs=wo_sb[:, h, :],
                                start=(h == 0), stop=(h == 1),
                                skip_group_check=True,
                            )
                    for j in range(2):
                        nt = g * 2 + j
                        nc.vector.tensor_add(
                            out0_sb[:, nt, :], pr0[:, j * 512:(j + 1) * 512],
                            bo_bc[:])

            # ---- passes: pass 0 with production pipelined one chunk
            # ahead; pass 1 pure attention from resident kt_f1/v_full.
            # Per pass: QK(mi) per m-tile; an exp tile is emitted as soon as
            # its 1536 rows of scores are complete; AVs for exp tile k-1 are
            # emitted after exp k (so the exp wait never blocks QK). ----
            def attention(p, mi, st):
                if (p, mi) in pre:
                    e_t = pre.pop((p, mi))
                else:
                    e_t = qk_exp(p, mi)
                st["pend"].append((mi, e_t))
                # AVs lag so the 3-op DVE exp chain has time to materialize
                # its E tile before the PE reaches the AV
                lag = 4 if p == 1 else 3
                while len(st["pend"]) > lag:
                    m_, e_ = st["pend"].pop(0)
                    av(p, m_, e_, st["acc"])

            kt1_fifo = []
            for p in range(2):
                acc = [accp.tile([128, 4, 128], f32, tag=f"acc{h2}",
                                 name=f"a{p}{h2}")
                       for h2 in range(2)]
                for h2 in range(2):
                    nc.vector.memset(acc[h2][:, :, 0:DIM_HEAD + 1], 0.0)
                st = {"pend": [], "acc": acc}
                if p == 1:
                    for c in range(4, len(CHUNKS)):
                        kt1_fifo.extend(produce_kt1(c))
                for step in range(len(CHUNKS) + 1):
                    if p == 1:
                        prod = kt1_fifo[:2]
                        del kt1_fifo[:2]
                    elif step == 0:
                        prod = chunk0[:-1]   # v of chunk 0
                    elif step == 1:
                        prod = produce_chunk(step) + [chunk0[-1]]
                    elif step < len(CHUNKS):
                        prod = produce_chunk(step)
                    else:
                        prod = []
                    # pass 0 attends chunk step-1 (produced one step ago);
                    # pass 1 is fully resident so it attends chunk `step`
                    # with no lag.
                    ac = step - 1 if p == 0 else step
                    if 0 <= ac < len(CHUNKS):
                        pm0, pmlen = CHUNKS[ac]
                        atts = list(range(pm0 // 128, (pm0 + pmlen) // 128))
                    else:
                        atts = []
                    for i in range(max(2 * len(prod), len(atts))):
                        if i < len(atts):
                            attention(p, atts[i], st)
                        if p == 0:
                            if i % 2 == 0 and i // 2 < len(prod):
                                prod[i // 2]()
                        else:
                            # deferred kt1 pops late in the step so the ct
                            # re-DMA has landed before the PE reaches them
                            if prod and i == 2:
                                prod.pop(0)()
                            elif prod and i == min(5, max(3, len(atts) - 1)):
                                prod.pop(0)()
                        if p == 0 and step == 4 and i == 0:
                            # mid-kernel constants, ordered behind the early
                            # ct chunks on the DMA queue
                            nc.sync.dma_start(out=ident_sb[:], in_=ident[:, :])
                            nc.sync.dma_start(out=wo_sb[:], in_=wo[:, :, :])
                            nc.sync.dma_start(out=bo_sb[:], in_=bo2[:, :])
                            nc.gpsimd.partition_broadcast(
                                bo_bc[:], bo_sb[0:1, :])
                        if p == 1 and step == 2 and i in (0, 4):
                            proj_pair0(i // 4)
                for m_, e_ in st["pend"]:
                    av(p, m_, e_, acc)
                if p == 0:
                    # kt1 for chunks 2-3: DMAs issued now so they are in
                    # flight across the pass boundary
                    kt1_fifo.extend(produce_kt1(2) + produce_kt1(3))
                    # pre-issue pass-1's first QK/exps so the pass boundary
                    # has no ACT bubble
                    for mi in range(5):
                        pre[(1, mi)] = qk_exp(1, mi)
                if p == 0:
                    tps = pass_tail(p, acc)
                    for h2 in range(2):
                        nc.vector.tensor_copy(
                            stack_sb[:, h2, :], tps[h2][:, :])

            # ---- pass-1 tail fused with the final projection, per 2-n-tile
            # group: normalize + transpose + stack slices, then the
            # projection PSUM is seeded with pair-0's result (identity
            # matmul), pair-1 accumulates on top, PSUM -> SBUF on the (now
            # idle) Activation engine, store. ----
            tp1 = [accp.tile([DIM_HEAD, N], bf16, tag=f"acc{h2}",
                             name=f"tp1{h2}") for h2 in range(2)]
            for g in range(2):
                for h2 in range(2):
                    nc.vector.reciprocal(
                        recip_sb[:, h2, 2 * g:2 * g + 2],
                        acc[h2][:, 2 * g:2 * g + 2, DIM_HEAD],
                    )
                    for j in range(2):
                        nt = g * 2 + j
                        nc.vector.tensor_scalar_mul(
                            norm_sb[:, h2, nt, :],
                            acc[h2][:, nt, 0:DIM_HEAD],
                            recip_sb[:, h2, nt:nt + 1],
                        )
                        nc.tensor.transpose(
                            tp1[h2][:, nt * 128:(nt + 1) * 128],
                            norm_sb[:, h2, nt, :],
                            ident_sb[:],
                        )
                    nc.vector.tensor_copy(
                        stack_sb[:, 2 + h2, g * 256:(g + 1) * 256],
                        tp1[h2][:, g * 256:(g + 1) * 256])
                pr = scp.tile([128, 1024], f32, tag="sc", name=f"pr{g}")
                for j in range(2):
                    nt = g * 2 + j
                    nc.tensor.matmul(
                        pr[:, j * 512:(j + 1) * 512],
                        lhsT=ident_sb[:],
                        rhs=out0_sb[:, nt, :],
                        start=True, stop=False,
                        skip_group_check=True,
                    )
                    for h in range(2, 4):
                        nc.tensor.matmul(
                            pr[:, j * 512:(j + 1) * 512],
                            lhsT=stack_sb[:, h, nt * 128:(nt + 1) * 128],
                            rhs=wo_sb[:, h, :],
                            start=False, stop=(h == 3),
                            skip_group_check=True,
                        )
                    nc.scalar.copy(out_sb[:, nt, :],
                                   pr[:, j * 512:(j + 1) * 512])
                    nc.sync.dma_start(out=out_r[:, nt, :],
                                      in_=out_sb[:, nt, :])

    nc.compile()
    return nc


def _get_nc():
    if "nc" not in _CACHE:
        _CACHE["nc"] = _build_nc()
    return _CACHE["nc"]


def _make_in_maps(x, context, Wq, Wkv, Wo, bo):
    import ml_dtypes
    bf = ml_dtypes.bfloat16

    x = np.asarray(x, dtype=np.float32)
    context = np.asarray(context, dtype=np.float32)
    Wq = np.asarray(Wq, dtype=np.float32)
    Wkv = np.asarray(Wkv, dtype=np.float32)
    Wo = np.asarray(Wo, dtype=np.float32)
    bo = np.asarray(bo, dtype=np.float32)

    Wk = Wkv[:, :ATT_DIM]
    Wv = Wkv[:, ATT_DIM:]
    bo2 = np.ascontiguousarray((bo / 2.0)[None, :])
    ident = np.eye(128, dtype=bf)

    in_maps = []
    for c in range(N_CORES):
        b, g = divmod(c, 2)
        hs = g * HPC * DIM_HEAD           # column offset of this core's heads
        he = hs + HPC * DIM_HEAD
        wo_core = Wo[hs:he, :].reshape(HPC, DIM_HEAD, QUERY_DIM)
        in_maps.append({
            "ct": np.ascontiguousarray(context[b].T.astype(bf)),
            "xt": np.ascontiguousarray(x[b].T.astype(bf)),
            "wq": np.ascontiguousarray(Wq[:, hs:he].astype(bf)),
            "wk": np.ascontiguousarray(Wk[:, hs:he].astype(bf)),
            "wv": np.ascontiguousarray(Wv[:, hs:he].astype(bf)),
            "wo": np.ascontiguousarray(wo_core.transpose(1, 0, 2)),
            "bo2": bo2,
            "ident": ident,
        })
    return in_maps


def run(inputs, trace=False, **spmd_kwargs):
    """Run the kernel; returns (full_output [B,N,QUERY_DIM], BassKernelResults)."""
    from concourse.bass_utils import run_bass_kernel_spmd

    nc = _get_nc()
    in_maps = _make_in_maps(**inputs)
    res = run_bass_kernel_spmd(
        nc, in_maps, core_ids=list(range(N_CORES)), trace=trace, **spmd_kwargs
    )
    outs = [np.asarray(r["out"], dtype=np.float32) for r in res.results]
    full = np.empty((B, N, QUERY_DIM), dtype=np.float32)
    for b in range(B):
        full[b] = outs[2 * b] + outs[2 * b + 1]
    return full, res


def kernel(**inputs) -> np.ndarray:
    full, _ = run(inputs, trace=False)
    return full



# revision 57
# speedup vs baseline: 1.0209x; 1.0209x over previous
# Bass/Tile Trainium2 kernel for nn_Attention_48816598286380.
#
# Reference computation (B=4, N=512, M=8192, Hq=512, Ck=256, H=8, D=64):
#   q = x @ Wq;  k,v = split(context @ Wkv);  per-head softmax(q k^T / sqrt(D)) v
#   out = attn_out @ Wo + bo
#
# Sharding: 8 cores = 4 batches x 2 head-groups (4 heads each).  Each core
# computes its batch's attention for its 4 heads plus the partial output
# projection over those heads; the host sums the two partial projections per
# batch (bo is split half/half so the sum carries the full bias).
#
# The kernel is Activation-engine bound: every score element passes through
# ACT exp exactly once (16.8M elements/core = 131072 rows at 128 lanes,
# ~0.85ns/row + ~0.19us/instr overhead), so the design keeps ACT dense from
# t~8us and pulls all other engines under it:
#   all matmul inputs are bf16 (host-converted; fp32 accumulation in PSUM)
#     which halves every input DMA at unchanged PE cost
#   scoresT[m, (h n)] = kT(m-tile).T @ qT  (two heads per PSUM tile)
#   E = exp(scoresT/8) on ACT, PSUM -> SBUF bf16
#   AV flipped: acc[n-tile, d_aug] += E-tile.T @ v_aug  (bf16, J=65; v_aug =
#     [v | ones] so the softmax denominator falls out of column 64).  AV
#     lags QK by one m-tile so the exp wait never blocks the next QK, and
#     all AV matmuls accumulate onto DVE-zeroed PSUM: interleaved
#     start=True groups within one PSUM bank corrupt earlier unstopped
#     groups on real hardware.
#   tail: per-head reciprocal + per-n-tile scale (DVE), PE transpose via
#     identity back to [d, n]; the pair-1 projection accumulates onto PSUM
#     seeded with pair-0's result via an identity matmul, the (idle) ACT
#     engine copies PSUM->SBUF bf16, and the host sums the two partials.
# kT pair 1 stays resident in SBUF (bf16, 16KB/partition) so pass 1 has no
# production or ct DMA at all.

import numpy as np

B, N, M = 4, 512, 8192
QUERY_DIM, INPUT_DIM = 512, 256
HEADS, DIM_HEAD = 8, 64
ATT_DIM = HEADS * DIM_HEAD  # 512
HPC = 4          # heads per core
N_CORES = 8
# chunk schedule: two small chunks first so the first scores/exp start
# as early as possible, then full-size chunks
CHUNKS = [(0, 512), (512, 512)] + [(m0, 1024) for m0 in range(1024, M, 1024)]
MCHUNK = 1024    # max chunk size (pool slot size)
NMI = M // 128   # 64 m-tiles per pass
SCALE = DIM_HEAD ** -0.5

# Pass-1 is ACT-throughput-bound (no production to hide behind), so DVE_EXP
# of its 64 exp tiles run on the Vector engine via a calibrated Schraudolph
# pair: s1 = i16(round(A*s + B1)) (DVE converts f32->int round-to-nearest),
# s2 = s1 + 64 (exactly the half-octave-offset second factor),
# E = bf16(s1.bf16 * s2.bf16).  Ripple ~1% rms, mean calibrated to exp;
# softmax cancels most of it (measured end-to-end rel-err cost ~2e-4).
DVE_EXP = 19
DVE_MIN_MI = 6       # keep the pre-issued boundary tiles on ACT
# pass-0 DVE-exp tiles: one per 1024-chunk, at the chunk's second-to-last
# visit (the production-copy queue on DVE is empty there)
DVE0_TILES = frozenset(range(15, NMI, 8))
LOG2E = float(np.log2(np.e))
SCH_A = 128.0 * LOG2E * SCALE / 2.0
SCH_B1 = 128.0 * (127.0 - 0.30755)

_CACHE = {}


def _build_nc():
    import concourse.bacc as bacc
    import concourse.bass as bass
    import concourse.mybir as mybir
    import concourse.tile as tile

    f32 = mybir.dt.float32
    f32r = mybir.dt.float32r
    bf16 = mybir.dt.bfloat16
    i16 = mybir.dt.int16
    EXP = mybir.ActivationFunctionType.Exp
    ADD = mybir.AluOpType.add
    MULT = mybir.AluOpType.mult

    # spread the DVE-exp'd pass-1 tiles evenly over mi in [DVE_MIN_MI, NMI)
    dve_tile = {DVE_MIN_MI + (i * (NMI - DVE_MIN_MI)) // DVE_EXP
                for i in range(DVE_EXP)}

    nc = bacc.Bacc(None, target_bir_lowering=False)

    ct = nc.dram_tensor("ct", [INPUT_DIM, M], bf16, kind="ExternalInput")  # context[b].T
    xt = nc.dram_tensor("xt", [QUERY_DIM, N], bf16, kind="ExternalInput")  # x[b].T
    wq = nc.dram_tensor("wq", [QUERY_DIM, HPC * DIM_HEAD], bf16, kind="ExternalInput")
    wk = nc.dram_tensor("wk", [INPUT_DIM, HPC * DIM_HEAD], bf16, kind="ExternalInput")
    wv = nc.dram_tensor("wv", [INPUT_DIM, HPC * DIM_HEAD], bf16, kind="ExternalInput")
    wo = nc.dram_tensor("wo", [DIM_HEAD, HPC, QUERY_DIM], f32r, kind="ExternalInput")
    bo2 = nc.dram_tensor("bo2", [1, QUERY_DIM], f32, kind="ExternalInput")  # bo / 2
    ident = nc.dram_tensor("ident", [128, 128], bf16, kind="ExternalInput")
    out = nc.dram_tensor("out", [N, QUERY_DIM], bf16, kind="ExternalOutput")

    ct_r = ct[:, :].rearrange("(t p) m -> p t m", p=128)    # [128, 2, M]
    xt_r = xt[:, :].rearrange("(t p) n -> p t n", p=128)    # [128, 4, N]
    wq_r = wq[:, :].rearrange("(t p) d -> p t d", p=128)    # [128, 4, 256]
    wk_r = wk[:, :].rearrange("(t p) d -> p t d", p=128)    # [128, 2, 256]
    wv_r = wv[:, :].rearrange("(t p) d -> p t d", p=128)    # [128, 2, 256]
    out_r = out[:, :].rearrange("(t p) f -> p t f", p=128)  # [128, 4, 512]

    with tile.TileContext(nc) as tc:
        with (
            tc.tile_pool(name="const", bufs=1) as cp,
            tc.tile_pool(name="ctp", bufs=3) as ctp,
            tc.tile_pool(name="ktp", bufs=2) as ktp,
            tc.tile_pool(name="ep", bufs=12) as ep,
            tc.tile_pool(name="sp1", bufs=3) as sp1,
            tc.tile_pool(name="sp2", bufs=3) as sp2,
            tc.tile_pool(name="scp", bufs=3, space="PSUM") as scp,
            tc.tile_pool(name="accp", bufs=1, space="PSUM") as accp,
        ):
            # ---- constants ----
            xt_sb = cp.tile([128, 4, N], bf16)
            wq_sb = cp.tile([128, 4, HPC * DIM_HEAD], bf16)
            wk_sb = cp.tile([128, 2, HPC * DIM_HEAD], bf16)
            wv_sb = cp.tile([128, 2, HPC * DIM_HEAD], bf16)
            wo_sb = cp.tile([DIM_HEAD, HPC, QUERY_DIM], f32r)
            bo_sb = cp.tile([1, QUERY_DIM], f32)
            bo_bc = cp.tile([128, QUERY_DIM], f32)
            ident_sb = cp.tile([128, 128], bf16)
            qt_sb = cp.tile([128, 2, N], bf16)
            # v for all 4 heads, all of M, with a ones column per head:
            # [128 (m within tile), m-tile, head, 64 v | 1 one]  (bf16)
            v_full = cp.tile([128, NMI, HPC, DIM_HEAD + 1], bf16)
            kt_f1 = cp.tile([128, M], bf16)               # pair-1 kT, resident
            stack_sb = cp.tile([DIM_HEAD, HPC, N], f32r)  # normalized attn outT
            recip_sb = cp.tile([128, 2, 4], f32)          # per pass-head, n-tile
            norm_sb = cp.tile([128, 2, 4, DIM_HEAD], bf16)  # normalized [n,d]
            out0_sb = cp.tile([128, 4, QUERY_DIM], bf16)  # pair-0 proj + bias
            out_sb = cp.tile([128, 4, QUERY_DIM], bf16)

            # prologue DMAs, interleaved so the qT path (wq+xt, the longer
            # pole) and the kT path (wk+ct0) both finish as early as
            # possible; xt is split per t-tile so the qT matmuls accumulate
            # as tiles land.
            nc.sync.dma_start(out=wq_sb[:, :, 0:128], in_=wq_r[:, :, 0:128])
            nc.sync.dma_start(out=xt_sb[:, 0:2, :], in_=xt_r[:, 0:2, :])
            nc.sync.dma_start(out=wk_sb[:], in_=wk_r)
            nc.sync.dma_start(out=xt_sb[:, 2:4, :], in_=xt_r[:, 2:4, :])

            # PE warm-up: the HAM clock gate holds the PE at 1.2 GHz until
            # ~3.4 us of sustained activity.  Run throwaway matmuls on a
            # zeroed tile while the prologue DMAs are in flight.
            warm_sb = cp.tile([128, 64], f32)
            nc.vector.memset(warm_sb[:], 0.0)
            # acc tiles are padded to [128, 4, 128] (exactly one 2KB PSUM
            # bank) so each [:, nt, 0:65] accumulation region stays inside
            # one bank.
            warm_ps = accp.tile([128, 4, 128], f32, tag="acc0",
                                name="warm_ps")
            for w in range(15):
                nc.tensor.matmul(
                    warm_ps[0:64, 0, 0:64], lhsT=warm_sb[:], rhs=warm_sb[:],
                    start=True, stop=True, skip_group_check=True,
                )

            kt_of = {}
            pre = {}   # (p, mi) -> e_t issued ahead of schedule
            dve_stash = {}

            def produce_chunk(mc):
                """DMA chunk mc of contextT; kT for pair 0 goes to rotating
                chunk tiles, pair 1 to the resident kt_f1, v (all 4 heads)
                to v_full.  Returns emitter closures so production
                interleaves with attention tiles."""
                m0, mlen = CHUNKS[mc]
                ct_t = ctp.tile([128, 2, MCHUNK], bf16, tag="ct",
                                name=f"ct{mc}")
                ct_dma = nc.sync.dma_start(
                    out=ct_t[:, :, 0:mlen], in_=ct_r[:, :, m0:m0 + mlen]
                )
                if mc >= 1:
                    # keep the small prologue DMAs ahead of the chunk stream
                    for d in late_dmas:
                        tile.add_dep_helper(ct_dma.ins, d.ins, sync=False,
                                            reason="prologue before ct stream")
                kt_t = ktp.tile([128, MCHUNK], bf16, tag="kt", name=f"kt{mc}")
                for mi in range(m0 // 128, (m0 + mlen) // 128):
                    kt_of[mi] = (kt_t, mi * 128 - m0)
                halves = mlen // 512

                def kt_group(pp):
                    def go():
                        kt_ps = scp.tile([128, 1024], f32, tag="sc",
                                         name=f"ktps{pp}{mc}")
                        for h2 in range(halves):
                            for t in range(2):
                                nc.tensor.matmul(
                                    kt_ps[:, h2 * 512:(h2 + 1) * 512],
                                    lhsT=wk_sb[:, t, pp * 128:(pp + 1) * 128],
                                    rhs=ct_t[:, t, h2 * 512:(h2 + 1) * 512],
                                    start=(t == 0), stop=(t == 1),
                                    skip_group_check=True,
                                )
                        dst = (kt_t[:, 0:mlen] if pp == 0 else
                               kt_f1[:, m0:m0 + mlen])
                        nc.vector.tensor_copy(dst, kt_ps[:, 0:mlen])
                    return go

                def v_group(s4):
                    def go():
                        v_ps = scp.tile([128, 1024], f32, tag="sc",
                                        name=f"vps{mc}{s4}")
                        for q in range(4):
                            s = s4 * 4 + q
                            for t in range(2):
                                nc.tensor.matmul(
                                    v_ps[:, q * 256:(q + 1) * 256],
                                    lhsT=ct_t[:, t, s * 128:(s + 1) * 128],
                                    rhs=wv_sb[:, t, :],
                                    start=(t == 0), stop=(t == 1),
                                    skip_group_check=True,
                                )
                        nc.vector.tensor_copy(
                            v_full[:, m0 // 128 + s4 * 4:
                                   m0 // 128 + s4 * 4 + 4, :, 0:DIM_HEAD],
                            v_ps[:].rearrange("p (s h d) -> p s h d", s=4, h=HPC),
                        )
                    return go

                # order: pair-0 kT first (needed immediately), v next (needed
                # by AV shortly after), pair-1 kT last (pass 1 only).  For
                # chunks >= 2 the pair-1 kT production is DEFERRED into pass
                # 1 (produce_kt1): pass 0 is PE-bound, pass 1 has PE slack.
                ops = [kt_group(0)]
                ops += [v_group(s4) for s4 in range(halves)]
                if mc < 2:
                    ops.append(kt_group(1))
                return ops

            def produce_kt1(mc):
                """Pass-1 deferred pair-1 kT production for chunk mc: re-DMA
                the ct chunk (DMA is idle in pass 1) and emit per-512-col
                closures so the borrowed score-ring slots are held briefly."""
                m0, mlen = CHUNKS[mc]
                ct_t = ctp.tile([128, 2, MCHUNK], bf16, tag="ct",
                                name=f"ct1_{mc}")
                nc.sync.dma_start(out=ct_t[:, :, 0:mlen],
                                  in_=ct_r[:, :, m0:m0 + mlen])

                def kt1_half(h2):
                    def go():
                        kt_ps = scp.tile([128, 1024], f32, tag="sc",
                                         name=f"ktps1_{mc}_{h2}")
                        for t in range(2):
                            nc.tensor.matmul(
                                kt_ps[:, 0:512],
                                lhsT=wk_sb[:, t, 128:256],
                                rhs=ct_t[:, t, h2 * 512:(h2 + 1) * 512],
                                start=(t == 0), stop=(t == 1),
                                skip_group_check=True,
                            )
                        dst = kt_f1[:, m0 + h2 * 512:m0 + (h2 + 1) * 512]
                        nc.vector.tensor_copy(dst, kt_ps[:, 0:512])
                    return go

                return [kt1_half(h2) for h2 in range(mlen // 512)]

            def qk_exp(p, mi):
                sc = scp.tile([128, 1024], f32, tag="sc", name=f"sc{p}{mi}")
                ks, off = kt_of[mi] if p == 0 else (kt_f1, mi * 128)
                ks = ks[:, off:off + 128]
                # two heads in one PE pass via row tiling
                nc.tensor.matmul(sc[:, 0:512], lhsT=ks[0:64, :],
                                 rhs=qt_sb[0:64, p, :], start=True, stop=True)
                nc.tensor.matmul(sc[:, 512:1024], lhsT=ks[64:128, :],
                                 rhs=qt_sb[64:128, p, :], start=True, stop=True)
                e_t = ep.tile([128, 1024], bf16, tag="e", name=f"e{p}{mi}")
                if (p == 1 and mi in dve_tile) or (p == 0 and mi in DVE0_TILES):
                    s1 = sp1.tile([128, 1024], i16, tag="s1", name=f"s1{p}_{mi}")
                    s2 = sp2.tile([128, 1024], i16, tag="s2", name=f"s2{p}_{mi}")
                    nc.vector.tensor_scalar(s1[:], sc[:], SCH_A, SCH_B1,
                                            MULT, ADD)
                    nc.vector.tensor_scalar(s2[:], s1[:], 64, None, ADD)
                    nc.vector.tensor_tensor(e_t[:], s1[:].bitcast(bf16),
                                            s2[:].bitcast(bf16), MULT)
                else:
                    nc.scalar.activation(e_t[:], sc[:], EXP, scale=SCALE)
                return e_t

            def av(p, mi, e_t, acc):
                # flipped AV: weights = E n-tile (full 128 rows), stream the
                # 65 v_aug columns
                for h2 in range(2):
                    for nt in range(4):
                        nc.tensor.matmul(
                            acc[h2][:, nt, 0:DIM_HEAD + 1],
                            lhsT=e_t[:, h2 * 512 + nt * 128:
                                     h2 * 512 + (nt + 1) * 128],
                            rhs=v_full[:, mi, 2 * p + h2, :],
                            start=False, stop=(mi == NMI - 1),
                            skip_group_check=True,
                        )

            def pass_tail(p, acc):
                """acc[h2] is [128 n(tile), 4 nt, 65] raw numerators with the
                denominator in column 64.  Normalize rows with one reciprocal
                and one broadcast multiply per head, transpose each
                [128 n, 64 d] block back to [d, n] on the PE, and return the
                two [64, N] PSUM tiles.  Pass 0 parks the transposed tiles in
                the (just-read) acc banks so the ring stays free for pass 1;
                pass 1 parks them in ring slot 2 (free after the last exp)."""
                tps = []
                for h2 in range(2):
                    tp_ps = accp.tile([DIM_HEAD, N], bf16,
                                      tag=f"acc{h2}", name=f"tp{p}{h2}")
                    nc.vector.reciprocal(
                        recip_sb[:, h2, :],
                        acc[h2][:, :, DIM_HEAD],
                    )
                    for nt in range(4):
                        nc.vector.tensor_scalar_mul(
                            norm_sb[:, h2, nt, :],
                            acc[h2][:, nt, 0:DIM_HEAD],
                            recip_sb[:, h2, nt:nt + 1],
                        )
                        nc.tensor.transpose(
                            tp_ps[:, nt * 128:(nt + 1) * 128],
                            norm_sb[:, h2, nt, :],
                            ident_sb[:],
                        )
                    tps.append(tp_ps)
                return tps

            # chunk-0 context DMA goes out right behind wk; the second half
            # of wq (pair 1) follows
            chunk0 = produce_chunk(0)
            nc.sync.dma_start(out=wq_sb[:, :, 128:256], in_=wq_r[:, :, 128:256])

            # late prologue (not needed until mid-kernel)
            late_dmas = []
            late_dmas.append(nc.sync.dma_start(out=wv_sb[:], in_=wv_r))
            # ones column of v_aug: memset a [128, 1] column, then one
            # broadcast-copy into the strided ones slots
            ones_col = cp.tile([128, 1], bf16)
            nc.vector.memset(ones_col[:], 1.0)
            _oc, _vdst = bass.broadcast_tensor_aps(
                ones_col[:, :], v_full[:, :, :, DIM_HEAD].rearrange(
                    "p s h -> p (s h)")[:, None, :].rearrange("p o q -> p (o q)")
            )
            nc.vector.tensor_copy(_vdst, _oc)

            # qT pair 0 (matmuls + copy) first, then chunk-0 kT (whose ct
            # lands slightly later), then qT pair 1; the DVE copy order
            # (qt-p0, kt, qt-p1) matches what the first QK needs.
            q_ps = scp.tile([128, 1024], f32, tag="sc", name="q_ps")

            def q_pair(p):
                for t in range(4):
                    nc.tensor.matmul(
                        q_ps[:, p * 512:(p + 1) * 512],
                        lhsT=wq_sb[:, t, p * 128:(p + 1) * 128],
                        rhs=xt_sb[:, t, :],
                        start=(t == 0), stop=(t == 3),
                        skip_group_check=True,
                    )
                nc.vector.tensor_copy(
                    qt_sb[:, p, :], q_ps[:, p * 512:(p + 1) * 512])

            q_pair(0)
            chunk0[0]()
            chunk0 = chunk0[1:]
            q_pair(1)

            # partial projection for pair 0 (+ bias) overlaps pass 1
            def proj_pair0(g):
                if True:
                    pr0 = scp.tile([128, 1024], f32, tag="sc", name=f"pr0{g}")
                    for j in range(2):
                        nt = g * 2 + j
                        for h in range(2):
                            nc.tensor.matmul(
                                pr0[:, j * 512:(j + 1) * 512],
                                lhsT=stack_sb[:, h, nt * 128:(nt + 1) * 128],
                                rhs=wo_sb[:, h, :],
                                start=(h == 0), stop=(h == 1),
                                skip_group_check=True,
                            )
                    for j in range(2):
                        nt = g * 2 + j
                        nc.vector.tensor_add(
                            out0_sb[:, nt, :], pr0[:, j * 512:(j + 1) * 512],
                            bo_bc[:])

            # ---- passes: pass 0 with production pipelined one chunk
            # ahead; pass 1 pure attention from resident kt_f1/v_full.
            # Per pass: QK(mi) per m-tile; an exp tile is emitted as soon as
            # its 1536 rows of scores are complete; AVs for exp tile k-1 are
            # emitted after exp k (so the exp wait never blocks QK). ----
            def attention(p, mi, st):
                if (p, mi) in pre:
                    e_t = pre.pop((p, mi))
                else:
                    e_t = qk_exp(p, mi)
                st["pend"].append((mi, e_t))
                # AVs lag so the 3-op DVE exp chain has time to materialize
                # its E tile before the PE reaches the AV
                lag = 4 if p == 1 else 3
                while len(st["pend"]) > lag:
                    m_, e_ = st["pend"].pop(0)
                    av(p, m_, e_, st["acc"])

            kt1_fifo = []
            for p in range(2):
                acc = [accp.tile([128, 4, 128], f32, tag=f"acc{h2}",
                                 name=f"a{p}{h2}")
                       for h2 in range(2)]
                for h2 in range(2):
                    nc.vector.memset(acc[h2][:, :, 0:DIM_HEAD + 1], 0.0)
                st = {"pend": [], "acc": acc}
                if p == 1:
                    for c in range(4, len(CHUNKS)):
                        kt1_fifo.extend(produce_kt1(c))
                for step in range(len(CHUNKS) + 1):
                    if p == 1:
                        prod = kt1_fifo[:2]
                        del kt1_fifo[:2]
                    elif step == 0:
                        prod = chunk0[:-1]   # v of chunk 0
                    elif step == 1:
                        prod = produce_chunk(step) + [chunk0[-1]]
                    elif step < len(CHUNKS):
                        prod = produce_chunk(step)
                    else:
                        prod = []
                    # pass 0 attends chunk step-1 (produced one step ago);
                    # pass 1 is fully resident so it attends chunk `step`
                    # with no lag.
                    ac = step - 1 if p == 0 else step
                    if 0 <= ac < len(CHUNKS):
                        pm0, pmlen = CHUNKS[ac]
                        atts = list(range(pm0 // 128, (pm0 + pmlen) // 128))
                    else:
                        atts = []
                    for i in range(max(2 * len(prod), len(atts))):
                        if i < len(atts):
                            attention(p, atts[i], st)
                        if p == 0:
                            if i % 2 == 0 and i // 2 < len(prod):
                                prod[i // 2]()
                        else:
                            # deferred kt1 pops late in the step so the ct
                            # re-DMA has landed before the PE reaches them
                            if prod and i == 2:
                                prod.pop(0)()
                            elif prod and i == min(5, max(3, len(atts) - 1)):
                                prod.pop(0)()
                        if p == 0 and step == 4 and i == 0:
                            # mid-kernel constants, ordered behind the early
                            # ct chunks on the DMA queue
                            nc.sync.dma_start(out=ident_sb[:], in_=ident[:, :])
                            nc.sync.dma_start(out=wo_sb[:], in_=wo[:, :, :])
                            nc.sync.dma_start(out=bo_sb[:], in_=bo2[:, :])
                            nc.gpsimd.partition_broadcast(
                                bo_bc[:], bo_sb[0:1, :])
                        if p == 1 and step == 2 and i in (0, 4):
                            proj_pair0(i // 4)
                for m_, e_ in st["pend"]:
                    av(p, m_, e_, acc)
                if p == 0:
                    # kt1 for chunks 2-3: DMAs issued now so they are in
                    # flight across the pass boundary
                    kt1_fifo.extend(produce_kt1(2) + produce_kt1(3))
                    # pre-issue pass-1's first QK/exps so the pass boundary
                    # has no ACT bubble
                    for mi in range(5):
                        pre[(1, mi)] = qk_exp(1, mi)
                if p == 0:
                    tps = pass_tail(p, acc)
                    for h2 in range(2):
                        nc.vector.tensor_copy(
                            stack_sb[:, h2, :], tps[h2][:, :])

            # ---- pass-1 tail fused with the final projection, per 2-n-tile
            # group: normalize + transpose + stack slices, then the
            # projection PSUM is seeded with pair-0's result (identity
            # matmul), pair-1 accumulates on top, PSUM -> SBUF on the (now
            # idle) Activation engine, store. ----
            tp1 = [accp.tile([DIM_HEAD, N], bf16, tag=f"acc{h2}",
                             name=f"tp1{h2}") for h2 in range(2)]
            for g in range(2):
                for h2 in range(2):
                    nc.vector.reciprocal(
                        recip_sb[:, h2, 2 * g:2 * g + 2],
                        acc[h2][:, 2 * g:2 * g + 2, DIM_HEAD],
                    )
                    for j in range(2):
                        nt = g * 2 + j
                        nc.vector.tensor_scalar_mul(
                            norm_sb[:, h2, nt, :],
                            acc[h2][:, nt, 0:DIM_HEAD],
                            recip_sb[:, h2, nt:nt + 1],
                        )
                        nc.tensor.transpose(
                            tp1[h2][:, nt * 128:(nt + 1) * 128],
                            norm_sb[:, h2, nt, :],
                            ident_sb[:],
                        )
                    nc.vector.tensor_copy(
                        stack_sb[:, 2 + h2, g * 256:(g + 1) * 256],
                        tp1[h2][:, g * 256:(g + 1) * 256])
                pr = scp.tile([128, 1024], f32, tag="sc", name=f"pr{g}")
                for j in range(2):
                    nt = g * 2 + j
                    nc.tensor.matmul(
                        pr[:, j * 512:(j + 1) * 512],
                        lhsT=ident_sb[:],
                        rhs=out0_sb[:, nt, :],
                        start=True, stop=False,
                        skip_group_check=True,
                    )
                    for h in range(2, 4):
                        nc.tensor.matmul(
                            pr[:, j * 512:(j + 1) * 512],
                            lhsT=stack_sb[:, h, nt * 128:(nt + 1) * 128],
                            rhs=wo_sb[:, h, :],
                            start=False, stop=(h == 3),
                            skip_group_check=True,
                        )
                    nc.scalar.copy(out_sb[:, nt, :],
                                   pr[:, j * 512:(j + 1) * 512])
                    nc.sync.dma_start(out=out_r[:, nt, :],
                                      in_=out_sb[:, nt, :])

    nc.compile()
    return nc


def _get_nc():
    if "nc" not in _CACHE:
        _CACHE["nc"] = _build_nc()
    return _CACHE["nc"]


def _make_in_maps(x, context, Wq, Wkv, Wo, bo):
    import ml_dtypes
    bf = ml_dtypes.bfloat16

    x = np.asarray(x, dtype=np.float32)
    context = np.asarray(context, dtype=np.float32)
    Wq = np.asarray(Wq, dtype=np.float32)
    Wkv = np.asarray(Wkv, dtype=np.float32)
    Wo = np.asarray(Wo, dtype=np.float32)
    bo = np.asarray(bo, dtype=np.float32)

    Wk = Wkv[:, :ATT_DIM]
    Wv = Wkv[:, ATT_DIM:]
    bo2 = np.ascontiguousarray((bo / 2.0)[None, :])
    ident = np.eye(128, dtype=bf)

    in_maps = []
    for c in range(N_CORES):
        b, g = divmod(c, 2)
        hs = g * HPC * DIM_HEAD           # column offset of this core's heads
        he = hs + HPC * DIM_HEAD
        wo_core = Wo[hs:he, :].reshape(HPC, DIM_HEAD, QUERY_DIM)
        in_maps.append({
            "ct": np.ascontiguousarray(context[b].T.astype(bf)),
            "xt": np.ascontiguousarray(x[b].T.astype(bf)),
            "wq": np.ascontiguousarray(Wq[:, hs:he].astype(bf)),
            "wk": np.ascontiguousarray(Wk[:, hs:he].astype(bf)),
            "wv": np.ascontiguousarray(Wv[:, hs:he].astype(bf)),
            "wo": np.ascontiguousarray(wo_core.transpose(1, 0, 2)),
            "bo2": bo2,
            "ident": ident,
        })
    return in_maps


def run(inputs, trace=False, **spmd_kwargs):
    """Run the kernel; returns (full_output [B,N,QUERY_DIM], BassKernelResults)."""
    from concourse.bass_utils import run_bass_kernel_spmd

    nc = _get_nc()
    in_maps = _make_in_maps(**inputs)
    res = run_bass_kernel_spmd(
        nc, in_maps, core_ids=list(range(N_CORES)), trace=trace, **spmd_kwargs
    )
    outs = [np.asarray(r["out"], dtype=np.float32) for r in res.results]
    full = np.empty((B, N, QUERY_DIM), dtype=np.float32)
    for b in range(B):
        full[b] = outs[2 * b] + outs[2 * b + 1]
    return full, res


def kernel(**inputs) -> np.ndarray:
    full, _ = run(inputs, trace=False)
    return full



# revision 58
# speedup vs baseline: 1.0280x; 1.0069x over previous
# Bass/Tile Trainium2 kernel for nn_Attention_48816598286380.
#
# Reference computation (B=4, N=512, M=8192, Hq=512, Ck=256, H=8, D=64):
#   q = x @ Wq;  k,v = split(context @ Wkv);  per-head softmax(q k^T / sqrt(D)) v
#   out = attn_out @ Wo + bo
#
# Sharding: 8 cores = 4 batches x 2 head-groups (4 heads each).  Each core
# computes its batch's attention for its 4 heads plus the partial output
# projection over those heads; the host sums the two partial projections per
# batch (bo is split half/half so the sum carries the full bias).
#
# The kernel is Activation-engine bound: every score element passes through
# ACT exp exactly once (16.8M elements/core = 131072 rows at 128 lanes,
# ~0.85ns/row + ~0.19us/instr overhead), so the design keeps ACT dense from
# t~8us and pulls all other engines under it:
#   all matmul inputs are bf16 (host-converted; fp32 accumulation in PSUM)
#     which halves every input DMA at unchanged PE cost
#   scoresT[m, (h n)] = kT(m-tile).T @ qT  (two heads per PSUM tile)
#   E = exp(scoresT/8) on ACT, PSUM -> SBUF bf16
#   AV flipped: acc[n-tile, d_aug] += E-tile.T @ v_aug  (bf16, J=65; v_aug =
#     [v | ones] so the softmax denominator falls out of column 64).  AV
#     lags QK by one m-tile so the exp wait never blocks the next QK, and
#     all AV matmuls accumulate onto DVE-zeroed PSUM: interleaved
#     start=True groups within one PSUM bank corrupt earlier unstopped
#     groups on real hardware.
#   tail: per-head reciprocal + per-n-tile scale (DVE), PE transpose via
#     identity back to [d, n]; the pair-1 projection accumulates onto PSUM
#     seeded with pair-0's result via an identity matmul, the (idle) ACT
#     engine copies PSUM->SBUF bf16, and the host sums the two partials.
# kT pair 1 stays resident in SBUF (bf16, 16KB/partition) so pass 1 has no
# production or ct DMA at all.

import numpy as np

B, N, M = 4, 512, 8192
QUERY_DIM, INPUT_DIM = 512, 256
HEADS, DIM_HEAD = 8, 64
ATT_DIM = HEADS * DIM_HEAD  # 512
HPC = 4          # heads per core
N_CORES = 8
# chunk schedule: two small chunks first so the first scores/exp start
# as early as possible, then full-size chunks
CHUNKS = [(0, 512), (512, 512)] + [(m0, 1024) for m0 in range(1024, M, 1024)]
MCHUNK = 1024    # max chunk size (pool slot size)
NMI = M // 128   # 64 m-tiles per pass
SCALE = DIM_HEAD ** -0.5

# Pass-1 is ACT-throughput-bound (no production to hide behind), so DVE_EXP
# of its 64 exp tiles run on the Vector engine via a calibrated Schraudolph
# pair: s1 = i16(round(A*s + B1)) (DVE converts f32->int round-to-nearest),
# s2 = s1 + 64 (exactly the half-octave-offset second factor),
# E = bf16(s1.bf16 * s2.bf16).  Ripple ~1% rms, mean calibrated to exp;
# softmax cancels most of it (measured end-to-end rel-err cost ~2e-4).
DVE_EXP = 19
DVE_MIN_MI = 6       # keep the pre-issued boundary tiles on ACT
# pass-0 DVE-exp tiles: one per 1024-chunk, at the chunk's second-to-last
# visit (the production-copy queue on DVE is empty there)
DVE0_TILES = frozenset(range(14, NMI, 8))
LOG2E = float(np.log2(np.e))
SCH_A = 128.0 * LOG2E * SCALE / 2.0
SCH_B1 = 128.0 * (127.0 - 0.30755)

_CACHE = {}


def _build_nc():
    import concourse.bacc as bacc
    import concourse.bass as bass
    import concourse.mybir as mybir
    import concourse.tile as tile

    f32 = mybir.dt.float32
    f32r = mybir.dt.float32r
    bf16 = mybir.dt.bfloat16
    i16 = mybir.dt.int16
    EXP = mybir.ActivationFunctionType.Exp
    ADD = mybir.AluOpType.add
    MULT = mybir.AluOpType.mult

    # spread the DVE-exp'd pass-1 tiles evenly over mi in [DVE_MIN_MI, NMI)
    dve_tile = {DVE_MIN_MI + (i * (NMI - DVE_MIN_MI)) // DVE_EXP
                for i in range(DVE_EXP)}

    nc = bacc.Bacc(None, target_bir_lowering=False)

    ct = nc.dram_tensor("ct", [INPUT_DIM, M], bf16, kind="ExternalInput")  # context[b].T
    xt = nc.dram_tensor("xt", [QUERY_DIM, N], bf16, kind="ExternalInput")  # x[b].T
    wq = nc.dram_tensor("wq", [QUERY_DIM, HPC * DIM_HEAD], bf16, kind="ExternalInput")
    wk = nc.dram_tensor("wk", [INPUT_DIM, HPC * DIM_HEAD], bf16, kind="ExternalInput")
    wv = nc.dram_tensor("wv", [INPUT_DIM, HPC * DIM_HEAD], bf16, kind="ExternalInput")
    wo = nc.dram_tensor("wo", [DIM_HEAD, HPC, QUERY_DIM], f32r, kind="ExternalInput")
    bo2 = nc.dram_tensor("bo2", [1, QUERY_DIM], f32, kind="ExternalInput")  # bo / 2
    ident = nc.dram_tensor("ident", [128, 128], bf16, kind="ExternalInput")
    out = nc.dram_tensor("out", [N, QUERY_DIM], bf16, kind="ExternalOutput")

    ct_r = ct[:, :].rearrange("(t p) m -> p t m", p=128)    # [128, 2, M]
    xt_r = xt[:, :].rearrange("(t p) n -> p t n", p=128)    # [128, 4, N]
    wq_r = wq[:, :].rearrange("(t p) d -> p t d", p=128)    # [128, 4, 256]
    wk_r = wk[:, :].rearrange("(t p) d -> p t d", p=128)    # [128, 2, 256]
    wv_r = wv[:, :].rearrange("(t p) d -> p t d", p=128)    # [128, 2, 256]
    out_r = out[:, :].rearrange("(t p) f -> p t f", p=128)  # [128, 4, 512]

    with tile.TileContext(nc) as tc:
        with (
            tc.tile_pool(name="const", bufs=1) as cp,
            tc.tile_pool(name="ctp", bufs=3) as ctp,
            tc.tile_pool(name="ktp", bufs=2) as ktp,
            tc.tile_pool(name="ep", bufs=12) as ep,
            tc.tile_pool(name="sp1", bufs=3) as sp1,
            tc.tile_pool(name="sp2", bufs=3) as sp2,
            tc.tile_pool(name="scp", bufs=3, space="PSUM") as scp,
            tc.tile_pool(name="accp", bufs=1, space="PSUM") as accp,
        ):
            # ---- constants ----
            xt_sb = cp.tile([128, 4, N], bf16)
            wq_sb = cp.tile([128, 4, HPC * DIM_HEAD], bf16)
            wk_sb = cp.tile([128, 2, HPC * DIM_HEAD], bf16)
            wv_sb = cp.tile([128, 2, HPC * DIM_HEAD], bf16)
            wo_sb = cp.tile([DIM_HEAD, HPC, QUERY_DIM], f32r)
            bo_sb = cp.tile([1, QUERY_DIM], f32)
            bo_bc = cp.tile([128, QUERY_DIM], f32)
            ident_sb = cp.tile([128, 128], bf16)
            qt_sb = cp.tile([128, 2, N], bf16)
            # v for all 4 heads, all of M, with a ones column per head:
            # [128 (m within tile), m-tile, head, 64 v | 1 one]  (bf16)
            v_full = cp.tile([128, NMI, HPC, DIM_HEAD + 1], bf16)
            kt_f1 = cp.tile([128, M], bf16)               # pair-1 kT, resident
            stack_sb = cp.tile([DIM_HEAD, HPC, N], f32r)  # normalized attn outT
            recip_sb = cp.tile([128, 2, 4], f32)          # per pass-head, n-tile
            norm_sb = cp.tile([128, 2, 4, DIM_HEAD], bf16)  # normalized [n,d]
            out0_sb = cp.tile([128, 4, QUERY_DIM], bf16)  # pair-0 proj + bias
            out_sb = cp.tile([128, 4, QUERY_DIM], bf16)

            # prologue DMAs, interleaved so the qT path (wq+xt, the longer
            # pole) and the kT path (wk+ct0) both finish as early as
            # possible; xt is split per t-tile so the qT matmuls accumulate
            # as tiles land.
            nc.sync.dma_start(out=wq_sb[:, :, 0:128], in_=wq_r[:, :, 0:128])
            nc.sync.dma_start(out=xt_sb[:, 0:2, :], in_=xt_r[:, 0:2, :])
            nc.sync.dma_start(out=wk_sb[:], in_=wk_r)
            nc.sync.dma_start(out=xt_sb[:, 2:4, :], in_=xt_r[:, 2:4, :])

            # PE warm-up: the HAM clock gate holds the PE at 1.2 GHz until
            # ~3.4 us of sustained activity.  Run throwaway matmuls on a
            # zeroed tile while the prologue DMAs are in flight.
            warm_sb = cp.tile([128, 64], f32)
            nc.vector.memset(warm_sb[:], 0.0)
            # acc tiles are padded to [128, 4, 128] (exactly one 2KB PSUM
            # bank) so each [:, nt, 0:65] accumulation region stays inside
            # one bank.
            warm_ps = accp.tile([128, 4, 128], f32, tag="acc0",
                                name="warm_ps")
            for w in range(15):
                nc.tensor.matmul(
                    warm_ps[0:64, 0, 0:64], lhsT=warm_sb[:], rhs=warm_sb[:],
                    start=True, stop=True, skip_group_check=True,
                )

            kt_of = {}
            pre = {}   # (p, mi) -> e_t issued ahead of schedule
            dve_stash = {}

            def produce_chunk(mc):
                """DMA chunk mc of contextT; kT for pair 0 goes to rotating
                chunk tiles, pair 1 to the resident kt_f1, v (all 4 heads)
                to v_full.  Returns emitter closures so production
                interleaves with attention tiles."""
                m0, mlen = CHUNKS[mc]
                ct_t = ctp.tile([128, 2, MCHUNK], bf16, tag="ct",
                                name=f"ct{mc}")
                ct_dma = nc.sync.dma_start(
                    out=ct_t[:, :, 0:mlen], in_=ct_r[:, :, m0:m0 + mlen]
                )
                if mc >= 1:
                    # keep the small prologue DMAs ahead of the chunk stream
                    for d in late_dmas:
                        tile.add_dep_helper(ct_dma.ins, d.ins, sync=False,
                                            reason="prologue before ct stream")
                kt_t = ktp.tile([128, MCHUNK], bf16, tag="kt", name=f"kt{mc}")
                for mi in range(m0 // 128, (m0 + mlen) // 128):
                    kt_of[mi] = (kt_t, mi * 128 - m0)
                halves = mlen // 512

                def kt_group(pp):
                    def go():
                        kt_ps = scp.tile([128, 1024], f32, tag="sc",
                                         name=f"ktps{pp}{mc}")
                        for h2 in range(halves):
                            for t in range(2):
                                nc.tensor.matmul(
                                    kt_ps[:, h2 * 512:(h2 + 1) * 512],
                                    lhsT=wk_sb[:, t, pp * 128:(pp + 1) * 128],
                                    rhs=ct_t[:, t, h2 * 512:(h2 + 1) * 512],
                                    start=(t == 0), stop=(t == 1),
                                    skip_group_check=True,
                                )
                        dst = (kt_t[:, 0:mlen] if pp == 0 else
                               kt_f1[:, m0:m0 + mlen])
                        nc.vector.tensor_copy(dst, kt_ps[:, 0:mlen])
                    return go

                def v_group(s4):
                    def go():
                        v_ps = scp.tile([128, 1024], f32, tag="sc",
                                        name=f"vps{mc}{s4}")
                        for q in range(4):
                            s = s4 * 4 + q
                            for t in range(2):
                                nc.tensor.matmul(
                                    v_ps[:, q * 256:(q + 1) * 256],
                                    lhsT=ct_t[:, t, s * 128:(s + 1) * 128],
                                    rhs=wv_sb[:, t, :],
                                    start=(t == 0), stop=(t == 1),
                                    skip_group_check=True,
                                )
                        nc.vector.tensor_copy(
                            v_full[:, m0 // 128 + s4 * 4:
                                   m0 // 128 + s4 * 4 + 4, :, 0:DIM_HEAD],
                            v_ps[:].rearrange("p (s h d) -> p s h d", s=4, h=HPC),
                        )
                    return go

                # order: pair-0 kT first (needed immediately), v next (needed
                # by AV shortly after), pair-1 kT last (pass 1 only).  For
                # chunks >= 2 the pair-1 kT production is DEFERRED into pass
                # 1 (produce_kt1): pass 0 is PE-bound, pass 1 has PE slack.
                ops = [kt_group(0)]
                ops += [v_group(s4) for s4 in range(halves)]
                if mc < 2:
                    ops.append(kt_group(1))
                return ops

            def produce_kt1(mc):
                """Pass-1 deferred pair-1 kT production for chunk mc: re-DMA
                the ct chunk (DMA is idle in pass 1) and emit per-512-col
                closures so the borrowed score-ring slots are held briefly."""
                m0, mlen = CHUNKS[mc]
                ct_t = ctp.tile([128, 2, MCHUNK], bf16, tag="ct",
                                name=f"ct1_{mc}")
                nc.sync.dma_start(out=ct_t[:, :, 0:mlen],
                                  in_=ct_r[:, :, m0:m0 + mlen])

                def kt1_half(h2):
                    def go():
                        kt_ps = scp.tile([128, 1024], f32, tag="sc",
                                         name=f"ktps1_{mc}_{h2}")
                        for t in range(2):
                            nc.tensor.matmul(
                                kt_ps[:, 0:512],
                                lhsT=wk_sb[:, t, 128:256],
                                rhs=ct_t[:, t, h2 * 512:(h2 + 1) * 512],
                                start=(t == 0), stop=(t == 1),
                                skip_group_check=True,
                            )
                        dst = kt_f1[:, m0 + h2 * 512:m0 + (h2 + 1) * 512]
                        nc.vector.tensor_copy(dst, kt_ps[:, 0:512])
                    return go

                return [kt1_half(h2) for h2 in range(mlen // 512)]

            def qk_exp(p, mi):
                sc = scp.tile([128, 1024], f32, tag="sc", name=f"sc{p}{mi}")
                ks, off = kt_of[mi] if p == 0 else (kt_f1, mi * 128)
                ks = ks[:, off:off + 128]
                # two heads in one PE pass via row tiling
                nc.tensor.matmul(sc[:, 0:512], lhsT=ks[0:64, :],
                                 rhs=qt_sb[0:64, p, :], start=True, stop=True)
                nc.tensor.matmul(sc[:, 512:1024], lhsT=ks[64:128, :],
                                 rhs=qt_sb[64:128, p, :], start=True, stop=True)
                e_t = ep.tile([128, 1024], bf16, tag="e", name=f"e{p}{mi}")
                if (p == 1 and mi in dve_tile) or (p == 0 and mi in DVE0_TILES):
                    s1 = sp1.tile([128, 1024], i16, tag="s1", name=f"s1{p}_{mi}")
                    s2 = sp2.tile([128, 1024], i16, tag="s2", name=f"s2{p}_{mi}")
                    nc.vector.tensor_scalar(s1[:], sc[:], SCH_A, SCH_B1,
                                            MULT, ADD)
                    nc.vector.tensor_scalar(s2[:], s1[:], 64, None, ADD)
                    nc.vector.tensor_tensor(e_t[:], s1[:].bitcast(bf16),
                                            s2[:].bitcast(bf16), MULT)
                else:
                    nc.scalar.activation(e_t[:], sc[:], EXP, scale=SCALE)
                return e_t

            def av(p, mi, e_t, acc):
                # flipped AV: weights = E n-tile (full 128 rows), stream the
                # 65 v_aug columns
                for h2 in range(2):
                    for nt in range(4):
                        nc.tensor.matmul(
                            acc[h2][:, nt, 0:DIM_HEAD + 1],
                            lhsT=e_t[:, h2 * 512 + nt * 128:
                                     h2 * 512 + (nt + 1) * 128],
                            rhs=v_full[:, mi, 2 * p + h2, :],
                            start=False, stop=(mi == NMI - 1),
                            skip_group_check=True,
                        )

            def pass_tail(p, acc):
                """acc[h2] is [128 n(tile), 4 nt, 65] raw numerators with the
                denominator in column 64.  Normalize rows with one reciprocal
                and one broadcast multiply per head, transpose each
                [128 n, 64 d] block back to [d, n] on the PE, and return the
                two [64, N] PSUM tiles.  Pass 0 parks the transposed tiles in
                the (just-read) acc banks so the ring stays free for pass 1;
                pass 1 parks them in ring slot 2 (free after the last exp)."""
                tps = []
                for h2 in range(2):
                    tp_ps = accp.tile([DIM_HEAD, N], bf16,
                                      tag=f"acc{h2}", name=f"tp{p}{h2}")
                    nc.vector.reciprocal(
                        recip_sb[:, h2, :],
                        acc[h2][:, :, DIM_HEAD],
                    )
                    for nt in range(4):
                        nc.vector.tensor_scalar_mul(
                            norm_sb[:, h2, nt, :],
                            acc[h2][:, nt, 0:DIM_HEAD],
                            recip_sb[:, h2, nt:nt + 1],
                        )
                        nc.tensor.transpose(
                            tp_ps[:, nt * 128:(nt + 1) * 128],
                            norm_sb[:, h2, nt, :],
                            ident_sb[:],
                        )
                    tps.append(tp_ps)
                return tps

            # chunk-0 context DMA goes out right behind wk; the second half
            # of wq (pair 1) follows
            chunk0 = produce_chunk(0)
            nc.sync.dma_start(out=wq_sb[:, :, 128:256], in_=wq_r[:, :, 128:256])

            # late prologue (not needed until mid-kernel)
            late_dmas = []
            late_dmas.append(nc.sync.dma_start(out=wv_sb[:], in_=wv_r))
            # ones column of v_aug: memset a [128, 1] column, then one
            # broadcast-copy into the strided ones slots
            ones_col = cp.tile([128, 1], bf16)
            nc.vector.memset(ones_col[:], 1.0)
            _oc, _vdst = bass.broadcast_tensor_aps(
                ones_col[:, :], v_full[:, :, :, DIM_HEAD].rearrange(
                    "p s h -> p (s h)")[:, None, :].rearrange("p o q -> p (o q)")
            )
            nc.vector.tensor_copy(_vdst, _oc)

            # qT pair 0 (matmuls + copy) first, then chunk-0 kT (whose ct
            # lands slightly later), then qT pair 1; the DVE copy order
            # (qt-p0, kt, qt-p1) matches what the first QK needs.
            q_ps = scp.tile([128, 1024], f32, tag="sc", name="q_ps")

            def q_pair(p):
                for t in range(4):
                    nc.tensor.matmul(
                        q_ps[:, p * 512:(p + 1) * 512],
                        lhsT=wq_sb[:, t, p * 128:(p + 1) * 128],
                        rhs=xt_sb[:, t, :],
                        start=(t == 0), stop=(t == 3),
                        skip_group_check=True,
                    )
                nc.vector.tensor_copy(
                    qt_sb[:, p, :], q_ps[:, p * 512:(p + 1) * 512])

            q_pair(0)
            chunk0[0]()
            chunk0 = chunk0[1:]
            q_pair(1)

            # partial projection for pair 0 (+ bias) overlaps pass 1
            def proj_pair0(g):
                if True:
                    pr0 = scp.tile([128, 1024], f32, tag="sc", name=f"pr0{g}")
                    for j in range(2):
                        nt = g * 2 + j
                        for h in range(2):
                            nc.tensor.matmul(
                                pr0[:, j * 512:(j + 1) * 512],
                                lhsT=stack_sb[:, h, nt * 128:(nt + 1) * 128],
                                rhs=wo_sb[:, h, :],
                                start=(h == 0), stop=(h == 1),
                                skip_group_check=True,
                            )
                    for j in range(2):
                        nt = g * 2 + j
                        nc.vector.tensor_add(
                            out0_sb[:, nt, :], pr0[:, j * 512:(j + 1) * 512],
                            bo_bc[:])

            # ---- passes: pass 0 with production pipelined one chunk
            # ahead; pass 1 pure attention from resident kt_f1/v_full.
            # Per pass: QK(mi) per m-tile; an exp tile is emitted as soon as
            # its 1536 rows of scores are complete; AVs for exp tile k-1 are
            # emitted after exp k (so the exp wait never blocks QK). ----
            def attention(p, mi, st):
                if (p, mi) in pre:
                    e_t = pre.pop((p, mi))
                else:
                    e_t = qk_exp(p, mi)
                st["pend"].append((mi, e_t))
                # AVs lag so the 3-op DVE exp chain has time to materialize
                # its E tile before the PE reaches the AV
                lag = 4 if p == 1 else 3
                while len(st["pend"]) > lag:
                    m_, e_ = st["pend"].pop(0)
                    av(p, m_, e_, st["acc"])

            kt1_fifo = []
            for p in range(2):
                acc = [accp.tile([128, 4, 128], f32, tag=f"acc{h2}",
                                 name=f"a{p}{h2}")
                       for h2 in range(2)]
                for h2 in range(2):
                    nc.vector.memset(acc[h2][:, :, 0:DIM_HEAD + 1], 0.0)
                st = {"pend": [], "acc": acc}
                if p == 1:
                    for c in range(4, len(CHUNKS)):
                        kt1_fifo.extend(produce_kt1(c))
                for step in range(len(CHUNKS) + 1):
                    if p == 1:
                        prod = kt1_fifo[:2]
                        del kt1_fifo[:2]
                    elif step == 0:
                        prod = chunk0[:-1]   # v of chunk 0
                    elif step == 1:
                        prod = produce_chunk(step) + [chunk0[-1]]
                    elif step < len(CHUNKS):
                        prod = produce_chunk(step)
                    else:
                        prod = []
                    # pass 0 attends chunk step-1 (produced one step ago);
                    # pass 1 is fully resident so it attends chunk `step`
                    # with no lag.
                    ac = step - 1 if p == 0 else step
                    if 0 <= ac < len(CHUNKS):
                        pm0, pmlen = CHUNKS[ac]
                        atts = list(range(pm0 // 128, (pm0 + pmlen) // 128))
                    else:
                        atts = []
                    for i in range(max(2 * len(prod), len(atts))):
                        if i < len(atts):
                            attention(p, atts[i], st)
                        if p == 0:
                            if i % 2 == 0 and i // 2 < len(prod):
                                prod[i // 2]()
                        else:
                            # deferred kt1 pops late in the step so the ct
                            # re-DMA has landed before the PE reaches them
                            if prod and i == 2:
                                prod.pop(0)()
                            elif prod and i == min(5, max(3, len(atts) - 1)):
                                prod.pop(0)()
                        if p == 0 and step == 4 and i == 0:
                            # mid-kernel constants, ordered behind the early
                            # ct chunks on the DMA queue
                            nc.sync.dma_start(out=ident_sb[:], in_=ident[:, :])
                            nc.sync.dma_start(out=wo_sb[:], in_=wo[:, :, :])
                            nc.sync.dma_start(out=bo_sb[:], in_=bo2[:, :])
                            nc.gpsimd.partition_broadcast(
                                bo_bc[:], bo_sb[0:1, :])
                        if p == 1 and step == 2 and i in (0, 4):
                            proj_pair0(i // 4)
                for m_, e_ in st["pend"]:
                    av(p, m_, e_, acc)
                if p == 0:
                    # kt1 for chunks 2-3: DMAs issued now so they are in
                    # flight across the pass boundary
                    kt1_fifo.extend(produce_kt1(2) + produce_kt1(3))
                    # pre-issue pass-1's first QK/exps so the pass boundary
                    # has no ACT bubble
                    for mi in range(5):
                        pre[(1, mi)] = qk_exp(1, mi)
                if p == 0:
                    tps = pass_tail(p, acc)
                    for h2 in range(2):
                        nc.vector.tensor_copy(
                            stack_sb[:, h2, :], tps[h2][:, :])

            # ---- pass-1 tail fused with the final projection, per 2-n-tile
            # group: normalize + transpose + stack slices, then the
            # projection PSUM is seeded with pair-0's result (identity
            # matmul), pair-1 accumulates on top, PSUM -> SBUF on the (now
            # idle) Activation engine, store. ----
            tp1 = [accp.tile([DIM_HEAD, N], bf16, tag=f"acc{h2}",
                             name=f"tp1{h2}") for h2 in range(2)]
            for g in range(2):
                for h2 in range(2):
                    nc.vector.reciprocal(
                        recip_sb[:, h2, 2 * g:2 * g + 2],
                        acc[h2][:, 2 * g:2 * g + 2, DIM_HEAD],
                    )
                    for j in range(2):
                        nt = g * 2 + j
                        nc.vector.tensor_scalar_mul(
                            norm_sb[:, h2, nt, :],
                            acc[h2][:, nt, 0:DIM_HEAD],
                            recip_sb[:, h2, nt:nt + 1],
                        )
                        nc.tensor.transpose(
                            tp1[h2][:, nt * 128:(nt + 1) * 128],
                            norm_sb[:, h2, nt, :],
                            ident_sb[:],
                        )
                    nc.vector.tensor_copy(
                        stack_sb[:, 2 + h2, g * 256:(g + 1) * 256],
                        tp1[h2][:, g * 256:(g + 1) * 256])
                pr = scp.tile([128, 1024], f32, tag="sc", name=f"pr{g}")
                for j in range(2):
                    nt = g * 2 + j
                    nc.tensor.matmul(
                        pr[:, j * 512:(j + 1) * 512],
                        lhsT=ident_sb[:],
                        rhs=out0_sb[:, nt, :],
                        start=True, stop=False,
                        skip_group_check=True,
                    )
                    for h in range(2, 4):
                        nc.tensor.matmul(
                            pr[:, j * 512:(j + 1) * 512],
                            lhsT=stack_sb[:, h, nt * 128:(nt + 1) * 128],
                            rhs=wo_sb[:, h, :],
                            start=False, stop=(h == 3),
                            skip_group_check=True,
                        )
                    nc.scalar.copy(out_sb[:, nt, :],
                                   pr[:, j * 512:(j + 1) * 512])
                    nc.sync.dma_start(out=out_r[:, nt, :],
                                      in_=out_sb[:, nt, :])

    nc.compile()
    return nc


def _get_nc():
    if "nc" not in _CACHE:
        _CACHE["nc"] = _build_nc()
    return _CACHE["nc"]


def _make_in_maps(x, context, Wq, Wkv, Wo, bo):
    import ml_dtypes
    bf = ml_dtypes.bfloat16

    x = np.asarray(x, dtype=np.float32)
    context = np.asarray(context, dtype=np.float32)
    Wq = np.asarray(Wq, dtype=np.float32)
    Wkv = np.asarray(Wkv, dtype=np.float32)
    Wo = np.asarray(Wo, dtype=np.float32)
    bo = np.asarray(bo, dtype=np.float32)

    Wk = Wkv[:, :ATT_DIM]
    Wv = Wkv[:, ATT_DIM:]
    bo2 = np.ascontiguousarray((bo / 2.0)[None, :])
    ident = np.eye(128, dtype=bf)

    in_maps = []
    for c in range(N_CORES):
        b, g = divmod(c, 2)
        hs = g * HPC * DIM_HEAD           # column offset of this core's heads
        he = hs + HPC * DIM_HEAD
        wo_core = Wo[hs:he, :].reshape(HPC, DIM_HEAD, QUERY_DIM)
        in_maps.append({
            "ct": np.ascontiguousarray(context[b].T.astype(bf)),
            "xt": np.ascontiguousarray(x[b].T.astype(bf)),
            "wq": np.ascontiguousarray(Wq[:, hs:he].astype(bf)),
            "wk": np.ascontiguousarray(Wk[:, hs:he].astype(bf)),
            "wv": np.ascontiguousarray(Wv[:, hs:he].astype(bf)),
            "wo": np.ascontiguousarray(wo_core.transpose(1, 0, 2)),
            "bo2": bo2,
            "ident": ident,
        })
    return in_maps


def run(inputs, trace=False, **spmd_kwargs):
    """Run the kernel; returns (full_output [B,N,QUERY_DIM], BassKernelResults)."""
    from concourse.bass_utils import run_bass_kernel_spmd

    nc = _get_nc()
    in_maps = _make_in_maps(**inputs)
    res = run_bass_kernel_spmd(
        nc, in_maps, core_ids=list(range(N_CORES)), trace=trace, **spmd_kwargs
    )
    outs = [np.asarray(r["out"], dtype=np.float32) for r in res.results]
    full = np.empty((B, N, QUERY_DIM), dtype=np.float32)
    for b in range(B):
        full[b] = outs[2 * b] + outs[2 * b + 1]
    return full, res


def kernel(**inputs) -> np.ndarray:
    full, _ = run(inputs, trace=False)
    return full

